# revision 10
# baseline (speedup 1.0000x reference)
"""MoE layer (nn_MoELayer_6923487282556) on 8 Trainium2 cores.

Strategy (expert-parallel, per sharding hint):
  Host router: router logits + top-2 softmax combine weights computed on
    host in fp32 (exactly mirrors the reference's fp32 math; top-2/top-3
    logit gaps are ~3.7e-5, far above fp32 dot error, so the routing
    matches). Host gathers tokens per expert ("all-to-all" dispatch) and
    pads to the max expert load.
  Single device launch (expert-parallel FFN): core e owns expert e.
    Computes silu(x@gwT) * (x@uwT) @ dwT scaled by the combine weight, in
    fp16 with fp32 PSUM accumulation (~5e-4 relative error end-to-end).
    Output is shipped fp16 (adds ~1e-4 error, halves output DMA).
  Host: scatter-add the two expert contributions per token.

Everything is transposed ([feature, token] layout) so no on-device
transposes are needed anywhere. All DRAM operands are laid out on the
host so each SBUF partition reads long contiguous bursts.
"""

import numpy as np

import concourse.bass as bass
import concourse.tile as tile
from concourse import bacc, mybir
from concourse.bass_utils import run_bass_kernel_spmd

F32 = mybir.dt.float32
F16 = mybir.dt.float16
AF = mybir.ActivationFunctionType
ALU = mybir.AluOpType
AX = mybir.AxisListType

N_CORES = 8
B, L, D = 4, 4096, 2048
N = B * L            # 16384 tokens
E = 8                # experts (== cores)
H = 3072             # ffn hidden
KC = D // 128        # 16 contraction chunks over D
MC = H // 128        # 24 chunks over H
DC = D // 128        # 16 output chunks over D

# When set (by test.py) the launch's execution is wrapped with the axon
# NTFF profile hook and traces land in PROFILE_DIR/ffn.
PROFILE_DIR = None

_cache = {}


def _run(nc, in_maps, tag):
    core_ids = list(range(N_CORES))
    if PROFILE_DIR is None:
        return run_bass_kernel_spmd(nc, in_maps, core_ids).results
    import os
    from trn_agent_boot.trn_boot import _ntff_profile_via_ctypes

    hook = _ntff_profile_via_ctypes("/opt/axon/libaxon_pjrt.so")
    # warm-up (NEFF compile) outside the profiled region
    run_bass_kernel_spmd(nc, in_maps, core_ids)
    out_dir = os.path.join(PROFILE_DIR, tag)
    os.makedirs(out_dir, exist_ok=True)
    with hook(out_dir, [0]):
        res = run_bass_kernel_spmd(nc, in_maps, core_ids).results
    return res


def _tile_sizes(cap):
    """First tile small so the PE starts ~2us in; then 1024-token tiles
    (2 chunks of 512) double-buffered in SBUF."""
    k = (cap - 128) // 1024
    T0 = cap - 1024 * k
    tiles = [T0] + [1024] * k
    assert sum(tiles) == cap and 128 <= T0 <= 1151
    return tiles


def _chunks(T):
    # matmul free dim caps at 512 (one PSUM bank of fp32 output per MM)
    chunks = []
    rem = T
    while rem:
        c = min(512, rem)
        chunks.append(c)
        rem -= c
    return chunks


def _build_ffn(cap, tiles):
    """Per core (expert e):
      xg   [128, KC, cap] fp16  partition-blocked gathered x.T
      gw/uw [MC, 128, KC, 128] fp16  partition-blocked transposed weights
      dw   [DC, 128, MC, 128] fp16
      wrep [128, cap] f32  combine weights replicated over partitions
    Output outT [D, cap] fp16 = (combine_w * expert_out).T."""
    nc = bacc.Bacc("TRN2", target_bir_lowering=False, debug=False,
                   num_devices=N_CORES)
    xg = nc.dram_tensor("xg", [128, KC, cap], F16,
                        kind="ExternalInput").ap()
    gw = nc.dram_tensor("gw", [MC, 128, KC, 128], F16,
                        kind="ExternalInput").ap()
    uw = nc.dram_tensor("uw", [MC, 128, KC, 128], F16,
                        kind="ExternalInput").ap()
    dw = nc.dram_tensor("dw", [DC, 128, MC, 128], F16,
                        kind="ExternalInput").ap()
    wrep = nc.dram_tensor("wrep", [128, cap], F32, kind="ExternalInput").ap()
    outT = nc.dram_tensor("outT", [D, cap], F16, kind="ExternalOutput").ap()

    with tile.TileContext(nc) as tc:
        with (
            tc.tile_pool(name="xp", bufs=2) as xp,
            tc.tile_pool(name="gp", bufs=2) as gp,
            tc.tile_pool(name="up", bufs=2) as up,
            tc.tile_pool(name="dp", bufs=2) as dp,
            tc.tile_pool(name="hp", bufs=1) as hp,
            tc.tile_pool(name="sg", bufs=2) as sgp,
            tc.tile_pool(name="op", bufs=2) as opl,
            tc.tile_pool(name="wpl", bufs=2) as wpl,
            tc.tile_pool(name="ps", bufs=2, space="PSUM") as ps,
        ):
            t0 = 0
            for T in tiles:
                chunks = _chunks(T)
                # per-k x tiles: first matmuls only wait on one small DMA
                # instead of the whole x tile; bufs=2 prefetches tile t+1
                # during tile t's compute
                xt = [xp.tile([128, T], F16, tag=f"xt{k}", name=f"xt{k}")
                      for k in range(KC)]
                for k in range(KC):
                    nc.sync.dma_start(xt[k][:], xg[:, k, t0:t0 + T])
                wt = wpl.tile([128, T], F32, tag="wt")
                nc.sync.dma_start(wt[:], wrep[:, t0:t0 + T])
                # one h tile per <=512-token chunk keeps every tile under
                # 64KB/partition
                h_tiles = [hp.tile([128, MC, cl], F16, tag=f"h{ci}",
                                   name=f"h{ci}")
                           for ci, cl in enumerate(chunks)]
                for m in range(MC):
                    gw_t = gp.tile([128, KC, 128], F16, tag="gw")
                    nc.sync.dma_start(gw_t[:], gw[m])
                    uw_t = up.tile([128, KC, 128], F16, tag="uw")
                    nc.sync.dma_start(uw_t[:], uw[m])
                    c0 = 0
                    for ci, cl in enumerate(chunks):
                        gps = ps.tile([128, cl], F32, tag="gps", bufs=2)
                        for k in range(KC):
                            nc.tensor.matmul(
                                gps[:], gw_t[:, k, :], xt[k][:, c0:c0 + cl],
                                start=(k == 0), stop=(k == KC - 1))
                        ups = ps.tile([128, cl], F32, tag="ups", bufs=2)
                        for k in range(KC):
                            nc.tensor.matmul(
                                ups[:], uw_t[:, k, :], xt[k][:, c0:c0 + cl],
                                start=(k == 0), stop=(k == KC - 1))
                        sg = sgp.tile([128, cl], F16, tag="sg")
                        nc.scalar.activation(sg[:], gps[:], AF.Silu)
                        nc.vector.tensor_mul(h_tiles[ci][:, m, :],
                                             sg[:], ups[:])
                        c0 += cl
                for d in range(DC):
                    dw_t = dp.tile([128, MC, 128], F16, tag="dw")
                    nc.sync.dma_start(dw_t[:], dw[d])
                    c0 = 0
                    for ci, cl in enumerate(chunks):
                        ops = ps.tile([128, cl], F32, tag="ops")
                        for m in range(MC):
                            nc.tensor.matmul(
                                ops[:], dw_t[:, m, :],
                                h_tiles[ci][:, m, :],
                                start=(m == 0), stop=(m == MC - 1))
                        ot = opl.tile([128, cl], F16, tag="ot")
                        nc.vector.tensor_mul(ot[:], ops[:], wt[:, c0:c0 + cl])
                        nc.sync.dma_start(
                            outT[d * 128:(d + 1) * 128, t0 + c0:t0 + c0 + cl],
                            ot[:])
                        c0 += cl
                t0 += T
    nc.compile()
    return nc


def _pblock(a):
    """[R, C] with R = r*128 -> [128, r, C] so each SBUF partition reads
    contiguous data."""
    r = a.shape[0] // 128
    return np.ascontiguousarray(
        a.reshape(r, 128, a.shape[1]).transpose(1, 0, 2))


def kernel(x, router_w, gate_w, up_w, down_w):
    x = np.asarray(x, np.float32)
    router_w = np.asarray(router_w, np.float32)
    gate_w = np.asarray(gate_w, np.float32)
    up_w = np.asarray(up_w, np.float32)
    down_w = np.asarray(down_w, np.float32)

    x_flat = np.ascontiguousarray(x.reshape(N, D))

    # ---- host router (fp32, mirrors the reference's math) ----
    logits = x_flat @ router_w.T                       # [N, E] fp32
    lmax = logits.max(axis=1, keepdims=True)
    probs = np.exp(logits - lmax, dtype=np.float32)
    probs /= probs.sum(axis=1, keepdims=True)
    top2 = np.argsort(-probs, axis=1, kind="stable")[:, :2]
    pk = np.take_along_axis(probs, top2, 1)
    pk = pk / pk.sum(axis=1, keepdims=True)
    combine = np.zeros((N, E), np.float32)
    np.put_along_axis(combine, top2, pk, 1)

    # ---- host dispatch: token lists per expert, padded to capacity ----
    idx = [np.flatnonzero(combine[:, e] > 0.0) for e in range(E)]
    max_cnt = max(len(i) for i in idx)
    cap = max(512, max_cnt)
    tiles = _tile_sizes(cap)

    gw16 = gate_w.astype(np.float16)
    uw16 = up_w.astype(np.float16)
    dw16 = down_w.astype(np.float16)
    x16 = x_flat.astype(np.float16)
    in_maps = []
    for e in range(E):
        cnt = len(idx[e])
        xg = np.zeros((D, cap), np.float16)
        xg[:, :cnt] = x16[idx[e]].T
        wvec = np.zeros(cap, np.float32)
        wvec[:cnt] = combine[idx[e], e]
        wrep = np.ascontiguousarray(np.broadcast_to(wvec, (128, cap)))
        # gw[m, p, k, c] = gate_w[e][m*128+c, k*128+p]
        gwb = np.ascontiguousarray(
            gw16[e].reshape(MC, 128, KC, 128).transpose(0, 3, 2, 1))
        uwb = np.ascontiguousarray(
            uw16[e].reshape(MC, 128, KC, 128).transpose(0, 3, 2, 1))
        # dw[d, p, m, c] = down_w[e][d*128+c, m*128+p]
        dwb = np.ascontiguousarray(
            dw16[e].reshape(DC, 128, MC, 128).transpose(0, 3, 2, 1))
        in_maps.append({"xg": _pblock(xg), "gw": gwb, "uw": uwb, "dw": dwb,
                        "wrep": wrep})

    key = ("ffn", cap)
    if key not in _cache:
        _cache[key] = _build_ffn(cap, tiles)
    nc_f = _cache[key]
    res_f = _run(nc_f, in_maps, "ffn")

    # ---- host scatter-add ("all-to-all" return) ----
    out = np.zeros((N, D), np.float32)
    for e in range(E):
        cnt = len(idx[e])
        if cnt:
            out[idx[e]] += res_f[e]["outT"][:, :cnt].T.astype(np.float32)
    return out.reshape(B, L, D)


# revision 11
# speedup vs baseline: 1.0719x; 1.0719x over previous
"""MoE layer (nn_MoELayer_6923487282556) on 8 Trainium2 cores.

Strategy (expert-parallel with balanced token slicing):
  Host router: router logits + top-2 softmax combine weights computed on
    host in fp32 (mirrors the reference's fp32 math; top-2/top-3 logit
    gaps are ~3.7e-5, far above fp32 dot error, so routing matches).
  Single device launch: each expert's token list is padded to a multiple
    of 8 and split into 8 equal slices; core c processes slice c of
    EVERY expert (8 segments, experts in a fixed size-sorted order).
    Per-core load = sum(cnt_e)/8 ~ 4099 tokens vs 4255 for pad-to-max
    expert-per-core. Weights are identical across cores (all 8 experts
    stream through each core, one expert per segment); only x/combine
    differ per core. FFN in fp16 with fp32 PSUM accumulation.
  Host: scatter-add the two expert contributions per token.

Everything is transposed ([feature, token] layout) so no on-device
transposes are needed anywhere."""

import numpy as np

import concourse.bass as bass
import concourse.tile as tile
from concourse import bacc, mybir
from concourse.bass_utils import run_bass_kernel_spmd

F32 = mybir.dt.float32
F16 = mybir.dt.float16
AF = mybir.ActivationFunctionType

N_CORES = 8
B, L, D = 4, 4096, 2048
N = B * L            # 16384 tokens
E = 8                # experts
H = 3072             # ffn hidden
KC = D // 128        # 16 contraction chunks over D
MC = H // 128        # 24 chunks over H
DC = D // 128        # 16 output chunks over D

PROFILE_DIR = None

_cache = {}


def _run(nc, in_maps, tag):
    core_ids = list(range(N_CORES))
    if PROFILE_DIR is None:
        return run_bass_kernel_spmd(nc, in_maps, core_ids).results
    import os
    from trn_agent_boot.trn_boot import _ntff_profile_via_ctypes

    hook = _ntff_profile_via_ctypes("/opt/axon/libaxon_pjrt.so")
    run_bass_kernel_spmd(nc, in_maps, core_ids)  # warm-up compile
    out_dir = os.path.join(PROFILE_DIR, tag)
    os.makedirs(out_dir, exist_ok=True)
    with hook(out_dir, [0]):
        res = run_bass_kernel_spmd(nc, in_maps, core_ids).results
    return res


def _seg_chunks(S):
    """Chunk a segment: single chunk if <=512, else two near halves
    (a 512+tiny-tail split wastes a dispatch-bound matmul group)."""
    if S <= 512:
        return [S]
    h = (S + 1) // 2
    return [h, S - h]


def _build_ffn(segs):
    """segs: list of segment sizes (per-core identical). Inputs per core:
      xg   [128, KC, cap] fp16   p-blocked x.T, segments concatenated
      gw/uw [E, MC, 128, KC, 128] fp16  weights per segment-rank
      dw   [E, DC, 128, MC, 128] fp16
      wrep [128, cap] f32
    Output outT [D, cap] f32."""
    cap = sum(segs)
    nc = bacc.Bacc("TRN2", target_bir_lowering=False, debug=False,
                   num_devices=N_CORES)
    xg = nc.dram_tensor("xg", [128, KC, cap], F16,
                        kind="ExternalInput").ap()
    gw = nc.dram_tensor("gw", [E, MC, 128, KC, 128], F16,
                        kind="ExternalInput").ap()
    uw = nc.dram_tensor("uw", [E, MC, 128, KC, 128], F16,
                        kind="ExternalInput").ap()
    dw = nc.dram_tensor("dw", [E, DC, 128, MC, 128], F16,
                        kind="ExternalInput").ap()
    wrep = nc.dram_tensor("wrep", [128, cap], F32, kind="ExternalInput").ap()
    outT = nc.dram_tensor("outT", [D, cap], F32, kind="ExternalOutput").ap()

    with tile.TileContext(nc) as tc:
        with (
            tc.tile_pool(name="xp", bufs=2) as xp,
            tc.tile_pool(name="gp", bufs=3) as gp,
            tc.tile_pool(name="up", bufs=3) as up,
            tc.tile_pool(name="dp", bufs=3) as dp,
            tc.tile_pool(name="hp", bufs=1) as hp,
            tc.tile_pool(name="sg", bufs=2) as sgp,
            tc.tile_pool(name="op", bufs=2) as opl,
            tc.tile_pool(name="wpl", bufs=2) as wpl,
            tc.tile_pool(name="ps", bufs=2, space="PSUM") as ps,
        ):
            t0 = 0
            for si, S in enumerate(segs):
                chunks = _seg_chunks(S)
                xt = [xp.tile([128, S], F16, tag=f"xt{k}", name=f"xt{k}")
                      for k in range(KC)]
                for k in range(KC):
                    nc.sync.dma_start(xt[k][:], xg[:, k, t0:t0 + S])
                wt = wpl.tile([128, S], F32, tag="wt")
                nc.sync.dma_start(wt[:], wrep[:, t0:t0 + S])
                h_tiles = [hp.tile([128, MC, cl], F16, tag=f"h{ci}",
                                   name=f"h{ci}")
                           for ci, cl in enumerate(chunks)]
                for m in range(MC):
                    gw_t = gp.tile([128, KC, 128], F16, tag="gw")
                    nc.sync.dma_start(gw_t[:, :KC // 2, :],
                                      gw[si, m, :, :KC // 2, :])
                    nc.sync.dma_start(gw_t[:, KC // 2:, :],
                                      gw[si, m, :, KC // 2:, :])
                    uw_t = up.tile([128, KC, 128], F16, tag="uw")
                    nc.sync.dma_start(uw_t[:, :KC // 2, :],
                                      uw[si, m, :, :KC // 2, :])
                    nc.sync.dma_start(uw_t[:, KC // 2:, :],
                                      uw[si, m, :, KC // 2:, :])
                    c0 = 0
                    for ci, cl in enumerate(chunks):
                        gps = ps.tile([128, cl], F32, tag="gps", bufs=2)
                        for k in range(KC):
                            nc.tensor.matmul(
                                gps[:], gw_t[:, k, :], xt[k][:, c0:c0 + cl],
                                start=(k == 0), stop=(k == KC - 1))
                        ups = ps.tile([128, cl], F32, tag="ups", bufs=2)
                        for k in range(KC):
                            nc.tensor.matmul(
                                ups[:], uw_t[:, k, :], xt[k][:, c0:c0 + cl],
                                start=(k == 0), stop=(k == KC - 1))
                        sg = sgp.tile([128, cl], F16, tag="sg")
                        nc.scalar.activation(sg[:], gps[:], AF.Silu)
                        nc.vector.tensor_mul(h_tiles[ci][:, m, :],
                                             sg[:], ups[:])
                        c0 += cl
                for d in range(DC):
                    dw_t = dp.tile([128, MC, 128], F16, tag="dw")
                    for q in range(4):
                        nc.sync.dma_start(
                            dw_t[:, q * (MC // 4):(q + 1) * (MC // 4), :],
                            dw[si, d, :, q * (MC // 4):(q + 1) * (MC // 4), :])
                    c0 = 0
                    for ci, cl in enumerate(chunks):
                        ops = ps.tile([128, cl], F32, tag="ops")
                        for m in range(MC):
                            nc.tensor.matmul(
                                ops[:], dw_t[:, m, :],
                                h_tiles[ci][:, m, :],
                                start=(m == 0), stop=(m == MC - 1))
                        ot = opl.tile([128, cl], F32, tag="ot")
                        nc.vector.tensor_mul(ot[:], ops[:], wt[:, c0:c0 + cl])
                        nc.sync.dma_start(
                            outT[d * 128:(d + 1) * 128, t0 + c0:t0 + c0 + cl],
                            ot[:])
                        c0 += cl
                t0 += S
    nc.compile()
    return nc


def _pblock(a):
    """[R, C] with R = r*128 -> [128, r, C]."""
    r = a.shape[0] // 128
    return np.ascontiguousarray(
        a.reshape(r, 128, a.shape[1]).transpose(1, 0, 2))


def kernel(x, router_w, gate_w, up_w, down_w):
    x = np.asarray(x, np.float32)
    router_w = np.asarray(router_w, np.float32)
    gate_w = np.asarray(gate_w, np.float32)
    up_w = np.asarray(up_w, np.float32)
    down_w = np.asarray(down_w, np.float32)

    x_flat = np.ascontiguousarray(x.reshape(N, D))

    # ---- host router (fp32, mirrors the reference's math) ----
    logits = x_flat @ router_w.T
    lmax = logits.max(axis=1, keepdims=True)
    probs = np.exp(logits - lmax, dtype=np.float32)
    probs /= probs.sum(axis=1, keepdims=True)
    top2 = np.argsort(-probs, axis=1, kind="stable")[:, :2]
    pk = np.take_along_axis(probs, top2, 1)
    pk = pk / pk.sum(axis=1, keepdims=True)
    combine = np.zeros((N, E), np.float32)
    np.put_along_axis(combine, top2, pk, 1)

    # ---- host dispatch: 8 equal slices per expert, slice c -> core c ----
    idx = [np.flatnonzero(combine[:, e] > 0.0) for e in range(E)]
    s_e = [(len(i) + 7) // 8 for i in idx]          # slice size per expert
    rank = sorted(range(E), key=lambda e: (s_e[e], e))  # canonical order,
    # smallest segment first (cheapest start-up x/weight DMA stall)
    segs = [s_e[e] for e in rank]
    cap = sum(segs)

    gw16 = gate_w.astype(np.float16)
    uw16 = up_w.astype(np.float16)
    dw16 = down_w.astype(np.float16)
    x16 = x_flat.astype(np.float16)

    # weights identical for all cores: blocked, in rank order
    gwb = np.empty((E, MC, 128, KC, 128), np.float16)
    uwb = np.empty((E, MC, 128, KC, 128), np.float16)
    dwb = np.empty((E, DC, 128, MC, 128), np.float16)
    for si, e in enumerate(rank):
        gwb[si] = gw16[e].reshape(MC, 128, KC, 128).transpose(0, 3, 2, 1)
        uwb[si] = uw16[e].reshape(MC, 128, KC, 128).transpose(0, 3, 2, 1)
        dwb[si] = dw16[e].reshape(DC, 128, MC, 128).transpose(0, 3, 2, 1)

    in_maps = []
    core_cols = []   # per core: list of (expert, token_ids, seg_off)
    for c in range(N_CORES):
        xgc = np.zeros((D, cap), np.float16)
        wvec = np.zeros(cap, np.float32)
        cols = []
        off = 0
        for si, e in enumerate(rank):
            s = segs[si]
            ids = idx[e][c * s:(c + 1) * s]
            xgc[:, off:off + len(ids)] = x16[ids].T
            wvec[off:off + len(ids)] = combine[ids, e]
            cols.append((e, ids, off))
            off += s
        wrep = np.ascontiguousarray(np.broadcast_to(wvec, (128, cap)))
        in_maps.append({"xg": _pblock(xgc), "gw": gwb, "uw": uwb,
                        "dw": dwb, "wrep": wrep})
        core_cols.append(cols)

    key = ("ffn", tuple(segs))
    if key not in _cache:
        _cache[key] = _build_ffn(segs)
    nc_f = _cache[key]
    res_f = _run(nc_f, in_maps, "ffn")

    # ---- host scatter-add ----
    out = np.zeros((N, D), np.float32)
    for c in range(N_CORES):
        oT = res_f[c]["outT"]
        for e, ids, off in core_cols[c]:
            if len(ids):
                out[ids] += oT[:, off:off + len(ids)].T
    return out.reshape(B, L, D)
